# revision 16
# baseline (speedup 1.0000x reference)
"""KANConv kernel for Trainium2 (8 NeuronCores, data-parallel over batch).

Math: out = conv2d_same(x, spline_weights.sum(-1)) + conv2d_same(silu(x), basis_weights)
    == conv2d_same(concat([x, silu(x)], ch), concat([w_spline, w_basis], cin))

Device strategy (per core, 2 images):
  - Host zero-pads x to (130, 130) fp16, precomputes silu(x) likewise, and
    folds the spline G-sum + weight concat to one (128cin, 9tap, 128cout)
    fp16 tensor.
  - Whole padded image resident in SBUF: tile [128p, 130, 130] fp16,
    partitions 0..63 = x, 64..127 = silu(x) (silu(0)=0 keeps the zero
    padding valid). Row chunks ride the two fast hardware-dynamic DMA
    queues (Sync=x, Scalar=silu) with nothing else ahead of them; the
    weights ride the otherwise-idle GpSimd SWDGE queue so their 300KB
    never delays the row stream. DVE-memset warmup matmuls hold the
    tensor engine's activity-managed clock through the prologue so the
    data stream starts warm.
  - Conv = 9 shifted matmuls accumulating in PSUM: per 4-output-row block j,
    psum[cout, 512] += w_tap[cin, cout].T @ x_shift[cin, 512]. fp16 operands
    stream at 1 row/cycle; LDWEIGHTS is fast-weight-load eligible and hides
    behind the previous stream.
  - PSUM tiles span 2 banks (2 j-blocks); DVE evacuates each pair with one
    fp32->fp16 cast copy; outputs DMA per 8-row block in fp16 (img0 on
    GpSimd, img1 on Sync), with the final 8 rows split 4+2+2 on Sync's
    then-idle queue to minimize the exposed tail. Host upcasts to fp32.
"""

import numpy as np

from concourse import bacc
import concourse.mybir as mybir
import concourse.tile as tile
from concourse.bass_utils import run_bass_kernel_spmd

B, CIN, COUT, H, W = 16, 64, 128, 128, 128
KH = KW = 3
G = 4
N_CORES = 8
B_LOC = B // N_CORES  # 2 images per core

P = 128           # partitions (= concat channel dim = cout)
HP, WP = H + 2, W + 2
FREE = 512        # psum free dim per j-block (fp32 bank)
RPM = FREE // W   # output rows per matmul/psum block = 4
NJ = H // RPM     # psum blocks per image = 32
JPD = 2           # psum blocks per psum tile / copy / output DMA (8 rows)

# x row chunks: small first chunks so the matmul stream starts early and
# per-chunk completion semaphores stay ahead of the consumption rate
# (~1.94us per 4 output rows)
ROWS0 = [(0, 6), (6, 14), (14, 24), (24, 40), (40, 72), (72, 104), (104, HP)]
ROWS1 = [(0, 32), (32, 64), (64, 96), (96, HP)]
# PE warmup matmuls during the prologue: the activity-managed clock needs
# >=3.4us of sustained full-array work to reach max. N=256 warmups take one
# warm-N=512 slot each at the cold clock, so they pace the ramp exactly.
# Sized so the warmup stream ends just before the chunk0/weights DMA
# completion gate (~11.6us) that releases the first data matmul.
N_WARM = 17
WARM_N = 256


def build_conv(tc, out_ap, xp_ap, sp_ap, w_ap, sink_ap):
    nc = tc.nc
    f16 = mybir.dt.float16
    f32 = mybir.dt.float32

    with (
        tc.tile_pool(name="wpool", bufs=1) as wpool,
        tc.tile_pool(name="xpool", bufs=2) as xpool,
        tc.tile_pool(name="opool", bufs=4) as opool,
        tc.tile_pool(name="psum", bufs=3, space="PSUM") as psum_pool,
        tc.tile_pool(name="psum_fin", bufs=1, space="PSUM") as psum_fin,
    ):
        # PE warmup: full-K/M matmuls keep the tensor engine busy until the
        # first data-gated matmul (partially-occupied or zero-data warmups
        # don't ramp the clock). Source comes from a DVE memset, which is
        # ready ~1.5us before gpsimd's first op would be. The psum result
        # is sunk to DRAM so the chain is observably live.
        # gpsimd exits the framework preamble first, so it seeds the warmup
        # source earliest (engine memset, not a DMA — the SWDGE queue stays
        # free for the weights)
        wscr = wpool.tile([P, WARM_N], f16, name="warm_src")
        nc.gpsimd.memset(wscr[:], 0.5)
        ptw_tile = psum_pool.tile([P, JPD, FREE], f32, name="ps")
        pt_w = ptw_tile[:, 0, :]
        for _ in range(N_WARM):
            nc.tensor.matmul(
                pt_w[:, :WARM_N], wscr[:, :P], wscr[:], start=True, stop=True
            )

        # DMA queue plan (FIFO per queue): x rows on Sync's hardware-dynamic
        # queue, silu rows on Scalar's; the weights ride GpSimd's SWDGE
        # queue, which is otherwise idle until the first img0 output DMA at
        # ~18us, so the row stream is never held up behind them.
        # split so tap0's completion (which gates the first data matmul)
        # lands ~1us earlier than a single 300KB transfer's; later taps
        # arrive just ahead of their consumption (213ns per tap)
        wt = wpool.tile([P, KH * KW, COUT], f16)
        nc.gpsimd.dma_start(out=wt[:, 0:1, :], in_=w_ap[:, 0:1, :])
        nc.gpsimd.dma_start(out=wt[:, 1:5, :], in_=w_ap[:, 1:5, :])
        nc.gpsimd.dma_start(out=wt[:, 5:9, :], in_=w_ap[:, 5:9, :])
        for img in range(B_LOC):
            xt = xpool.tile([P, HP, WP], f16)
            for r0, r1 in ROWS0 if img == 0 else ROWS1:
                nc.sync.dma_start(
                    out=xt[:CIN, r0:r1], in_=xp_ap[img, :, r0:r1, :]
                )
                nc.scalar.dma_start(
                    out=xt[CIN:, r0:r1], in_=sp_ap[img, :, r0:r1, :]
                )
            if img == 0:
                wsink = wpool.tile([1, 4], f32, name="warm_sink")
                nc.vector.tensor_copy(out=wsink[:], in_=pt_w[:1, :4])
                nc.scalar.dma_start(out=sink_ap, in_=wsink[:])
            last = img == B_LOC - 1
            for jj in range(0, NJ, JPD):
                if last and jj == NJ - JPD:
                    break
                ot = opool.tile([P, JPD * RPM, W], f16)
                pt = psum_pool.tile([P, JPD, FREE], f32, name="ps")
                for j in range(jj, jj + JPD):
                    for t in range(KH * KW):
                        dh, dw = t // KW, t % KW
                        rhs = xt[:, RPM * j + dh : RPM * j + dh + RPM, dw : dw + W]
                        nc.tensor.matmul(
                            pt[:, j - jj, :],
                            wt[:, t, :],
                            rhs,
                            start=(t == 0),
                            stop=(t == KH * KW - 1),
                        )
                nc.vector.tensor_copy(
                    out=ot[:],
                    in_=pt[:].rearrange("p j (r w) -> p (j r) w", w=W),
                )
                # img0 outputs ride GpSimd (Sync/Scalar queues busy with
                # inputs then); img1 outputs ride Sync's fast queue (its
                # input issues are long done by mid-stream)
                dma_eng = nc.gpsimd if img == 0 else nc.sync
                dma_eng.dma_start(
                    out=out_ap[img, :, jj * RPM : (jj + JPD) * RPM, :], in_=ot[:]
                )
        # final 2 j-blocks in progressively smaller pieces so the exposed
        # copy + DMA after the very last matmul is minimal
        img = B_LOC - 1
        j = NJ - 2
        ot = opool.tile([P, RPM, W], f16, name="ot_fin")
        pt = psum_pool.tile([P, JPD, FREE], f32, name="ps")
        for t in range(KH * KW):
            dh, dw = t // KW, t % KW
            rhs = xt[:, RPM * j + dh : RPM * j + dh + RPM, dw : dw + W]
            nc.tensor.matmul(
                pt[:, 0, :], wt[:, t, :], rhs, start=(t == 0), stop=(t == KH * KW - 1)
            )
        nc.vector.tensor_copy(
            out=ot[:], in_=pt[:, 0, :].rearrange("p (r w) -> p r w", w=W)
        )
        nc.sync.dma_start(
            out=out_ap[img, :, RPM * j : RPM * (j + 1), :], in_=ot[:]
        )
        # very last 4 output rows as two 2-row half-bank blocks
        for half in range(2):
            r = H - RPM + 2 * half
            oth = opool.tile([P, 2, W], f16, name=f"ot_h{half}")
            pth = psum_fin.tile([P, 2 * W], f32, name=f"ps_h{half}")
            for t in range(KH * KW):
                dh, dw = t // KW, t % KW
                rhs = xt[:, r + dh : r + dh + 2, dw : dw + W]
                nc.tensor.matmul(pth[:], wt[:, t, :], rhs, start=(t == 0), stop=(t == KH * KW - 1))
            nc.vector.tensor_copy(
                out=oth[:], in_=pth[:].rearrange("p (r w) -> p r w", w=W)
            )
            # scalar's HWDGE ring is idle by now — issuing the final pieces
            # there overlaps their issue latency with sync's j-block DMA
            nc.scalar.dma_start(out=out_ap[img, :, r : r + 2, :], in_=oth[:])


_CACHE = {}


def _get_nc():
    key = "nc"
    if key not in _CACHE:
        nc = bacc.Bacc("TRN2", target_bir_lowering=False, debug=False)
        xp = nc.dram_tensor(
            "xp", [B_LOC, CIN, HP, WP], mybir.dt.float16, kind="ExternalInput"
        ).ap()
        sp = nc.dram_tensor(
            "sp", [B_LOC, CIN, HP, WP], mybir.dt.float16, kind="ExternalInput"
        ).ap()
        w = nc.dram_tensor(
            "w", [P, KH * KW, COUT], mybir.dt.float16, kind="ExternalInput"
        ).ap()
        out = nc.dram_tensor(
            "out", [B_LOC, COUT, H, W], mybir.dt.float16, kind="ExternalOutput"
        ).ap()
        sink = nc.dram_tensor("warm_sink", [1, 4], mybir.dt.float32).ap()
        with tile.TileContext(nc) as tc:
            build_conv(tc, out, xp, sp, w, sink)
        nc.finalize()
        _CACHE[key] = nc
    return _CACHE[key]


def _prep_inputs(x, spline_weights, basis_weights):
    w_spline = spline_weights.sum(axis=-1)  # (COUT, CIN, 3, 3)
    w_cat = np.concatenate([w_spline, basis_weights], axis=1)  # (COUT, 128, 3, 3)
    # -> (cin_cat, tap, cout)
    w_ktm = np.ascontiguousarray(
        w_cat.transpose(1, 2, 3, 0).reshape(P, KH * KW, COUT).astype(np.float16)
    )
    x_pad = np.pad(x, ((0, 0), (0, 0), (1, 1), (1, 1))).astype(np.float16)
    s = (x * (1.0 / (1.0 + np.exp(-x)))).astype(np.float32)  # silu in fp32
    s_pad = np.pad(s, ((0, 0), (0, 0), (1, 1), (1, 1))).astype(np.float16)
    return x_pad, s_pad, w_ktm


def kernel(x, spline_weights, basis_weights, _trace=False, _tmpdir=None):
    x = np.asarray(x, dtype=np.float32)
    spline_weights = np.asarray(spline_weights, dtype=np.float32)
    basis_weights = np.asarray(basis_weights, dtype=np.float32)
    x_pad, s_pad, w_ktm = _prep_inputs(x, spline_weights, basis_weights)
    nc = _get_nc()
    in_maps = [
        {
            "xp": x_pad[B_LOC * c : B_LOC * (c + 1)],
            "sp": s_pad[B_LOC * c : B_LOC * (c + 1)],
            "w": w_ktm,
        }
        for c in range(N_CORES)
    ]
    res = run_bass_kernel_spmd(
        nc, in_maps, list(range(N_CORES)), trace=_trace, tmpdir=_tmpdir
    )
    out = np.concatenate(
        [res.results[c]["out"] for c in range(N_CORES)], axis=0
    ).astype(np.float32)
    if _trace:
        kernel.last_results = res
    return out


# revision 18
# speedup vs baseline: 1.0216x; 1.0216x over previous
"""KANConv kernel for Trainium2 (8 NeuronCores, data-parallel over batch).

Math: out = conv2d_same(x, spline_weights.sum(-1)) + conv2d_same(silu(x), basis_weights)
    == conv2d_same(concat([x, silu(x)], ch), concat([w_spline, w_basis], cin))

Device strategy (per core, 2 images):
  - Host zero-pads x to (130, 130) fp16, precomputes silu(x) likewise, and
    folds the spline G-sum + weight concat to one (128cin, 9tap, 128cout)
    fp16 tensor.
  - Whole padded image resident in SBUF: tile [128p, 130, 130] fp16,
    partitions 0..63 = x, 64..127 = silu(x) (silu(0)=0 keeps the zero
    padding valid). Row chunks ride the two fast hardware-dynamic DMA
    queues (Sync=x, Scalar=silu) with nothing else ahead of them; the
    weights ride the otherwise-idle GpSimd SWDGE queue so their 300KB
    never delays the row stream. DVE-memset warmup matmuls hold the
    tensor engine's activity-managed clock through the prologue so the
    data stream starts warm.
  - Conv = 9 shifted matmuls accumulating in PSUM: per 4-output-row block j,
    psum[cout, 512] += w_tap[cin, cout].T @ x_shift[cin, 512]. fp16 operands
    stream at 1 row/cycle; LDWEIGHTS is fast-weight-load eligible and hides
    behind the previous stream.
  - PSUM tiles span 2 banks (2 j-blocks); DVE evacuates each pair with one
    fp32->fp16 cast copy; outputs DMA per 8-row block in fp16 (img0 on
    GpSimd, img1 on Sync), with the final 8 rows split 4+2+2 on Sync's
    then-idle queue to minimize the exposed tail. Host upcasts to fp32.
"""

import numpy as np

from concourse import bacc
import concourse.mybir as mybir
import concourse.tile as tile
from concourse.bass_utils import run_bass_kernel_spmd

B, CIN, COUT, H, W = 16, 64, 128, 128, 128
KH = KW = 3
G = 4
N_CORES = 8
B_LOC = B // N_CORES  # 2 images per core

P = 128           # partitions (= concat channel dim = cout)
HP, WP = H + 2, W + 2
FREE = 512        # psum free dim per j-block (fp32 bank)
RPM = FREE // W   # output rows per matmul/psum block = 4
NJ = H // RPM     # psum blocks per image = 32
JPD = 2           # psum blocks per psum tile / copy / output DMA (8 rows)

# x row chunks: small first chunks so the matmul stream starts early and
# per-chunk completion semaphores stay ahead of the consumption rate
# (~1.94us per 4 output rows)
ROWS0 = [(0, 6), (6, 14), (14, 24), (24, 40), (40, 72), (72, 104), (104, HP)]
ROWS1 = [(0, 32), (32, 64), (64, 96), (96, HP)]
# PE warmup matmuls during the prologue: the activity-managed clock needs
# >=3.4us of sustained full-array work to reach max. N=256 warmups take one
# warm-N=512 slot each at the cold clock, so they pace the ramp exactly.
# Sized so the warmup stream ends just before the chunk0/weights DMA
# completion gate (~11.6us) that releases the first data matmul.
N_WARM = 24
WARM_N = 256


def build_conv(tc, out_ap, xp_ap, sp_ap, w_ap, sink_ap):
    nc = tc.nc
    f16 = mybir.dt.float16
    f32 = mybir.dt.float32

    with (
        tc.tile_pool(name="wpool", bufs=1) as wpool,
        tc.tile_pool(name="xpool", bufs=2) as xpool,
        tc.tile_pool(name="opool", bufs=4) as opool,
        tc.tile_pool(name="psum", bufs=3, space="PSUM") as psum_pool,
        tc.tile_pool(name="psum_fin", bufs=1, space="PSUM") as psum_fin,
    ):
        # PE warmup: full-K/M matmuls keep the tensor engine busy until the
        # first data-gated matmul (partially-occupied or zero-data warmups
        # don't ramp the clock). Source comes from a DVE memset, which is
        # ready ~1.5us before gpsimd's first op would be. The psum result
        # is sunk to DRAM so the chain is observably live.
        # gpsimd exits the framework preamble first, so it seeds the warmup
        # source earliest (engine memset, not a DMA — the SWDGE queue stays
        # free for the weights)
        wscr = wpool.tile([P, WARM_N], f16, name="warm_src")
        nc.gpsimd.memset(wscr[:], 0.5)
        ptw_tile = psum_pool.tile([P, JPD, FREE], f32, name="ps")
        pt_w = ptw_tile[:, 0, :]
        for _ in range(N_WARM):
            nc.tensor.matmul(
                pt_w[:, :WARM_N], wscr[:, :P], wscr[:], start=True, stop=True
            )

        # DMA queue plan (FIFO per queue): x rows on Sync's hardware-dynamic
        # queue, silu rows on Scalar's; the weights ride GpSimd's SWDGE
        # queue, which is otherwise idle until the first img0 output DMA at
        # ~18us, so the row stream is never held up behind them.
        # split in two so the first taps' completion (which gates the first
        # data matmul) lands ~0.7us earlier than a single 300KB transfer's
        wt = wpool.tile([P, KH * KW, COUT], f16)
        nc.gpsimd.dma_start(out=wt[:, 0:5, :], in_=w_ap[:, 0:5, :])
        nc.gpsimd.dma_start(out=wt[:, 5:9, :], in_=w_ap[:, 5:9, :])
        for img in range(B_LOC):
            xt = xpool.tile([P, HP, WP], f16)
            for r0, r1 in ROWS0 if img == 0 else ROWS1:
                nc.sync.dma_start(
                    out=xt[:CIN, r0:r1], in_=xp_ap[img, :, r0:r1, :]
                )
                nc.scalar.dma_start(
                    out=xt[CIN:, r0:r1], in_=sp_ap[img, :, r0:r1, :]
                )
            if img == 0:
                wsink = wpool.tile([1, 4], f32, name="warm_sink")
                nc.vector.tensor_copy(out=wsink[:], in_=pt_w[:1, :4])
                nc.scalar.dma_start(out=sink_ap, in_=wsink[:])
            last = img == B_LOC - 1
            for jj in range(0, NJ, JPD):
                if last and jj == NJ - JPD:
                    break
                ot = opool.tile([P, JPD * RPM, W], f16)
                pt = psum_pool.tile([P, JPD, FREE], f32, name="ps")
                for j in range(jj, jj + JPD):
                    for t in range(KH * KW):
                        dh, dw = t // KW, t % KW
                        rhs = xt[:, RPM * j + dh : RPM * j + dh + RPM, dw : dw + W]
                        nc.tensor.matmul(
                            pt[:, j - jj, :],
                            wt[:, t, :],
                            rhs,
                            start=(t == 0),
                            stop=(t == KH * KW - 1),
                        )
                nc.vector.tensor_copy(
                    out=ot[:],
                    in_=pt[:].rearrange("p j (r w) -> p (j r) w", w=W),
                )
                # img0 outputs ride GpSimd (Sync/Scalar queues busy with
                # inputs then); img1 outputs ride Sync's fast queue (its
                # input issues are long done by mid-stream)
                dma_eng = nc.gpsimd if img == 0 else nc.sync
                dma_eng.dma_start(
                    out=out_ap[img, :, jj * RPM : (jj + JPD) * RPM, :], in_=ot[:]
                )
        # final 2 j-blocks in progressively smaller pieces so the exposed
        # copy + DMA after the very last matmul is minimal
        img = B_LOC - 1
        j = NJ - 2
        ot = opool.tile([P, RPM, W], f16, name="ot_fin")
        pt = psum_pool.tile([P, JPD, FREE], f32, name="ps")
        for t in range(KH * KW):
            dh, dw = t // KW, t % KW
            rhs = xt[:, RPM * j + dh : RPM * j + dh + RPM, dw : dw + W]
            nc.tensor.matmul(
                pt[:, 0, :], wt[:, t, :], rhs, start=(t == 0), stop=(t == KH * KW - 1)
            )
        nc.vector.tensor_copy(
            out=ot[:], in_=pt[:, 0, :].rearrange("p (r w) -> p r w", w=W)
        )
        nc.sync.dma_start(
            out=out_ap[img, :, RPM * j : RPM * (j + 1), :], in_=ot[:]
        )
        # very last 4 output rows as two 2-row half-bank blocks
        for half in range(2):
            r = H - RPM + 2 * half
            oth = opool.tile([P, 2, W], f16, name=f"ot_h{half}")
            pth = psum_fin.tile([P, 2 * W], f32, name=f"ps_h{half}")
            for t in range(KH * KW):
                dh, dw = t // KW, t % KW
                rhs = xt[:, r + dh : r + dh + 2, dw : dw + W]
                nc.tensor.matmul(pth[:], wt[:, t, :], rhs, start=(t == 0), stop=(t == KH * KW - 1))
            nc.vector.tensor_copy(
                out=oth[:], in_=pth[:].rearrange("p (r w) -> p r w", w=W)
            )
            # scalar's HWDGE ring is idle by now — issuing the final pieces
            # there overlaps their issue latency with sync's j-block DMA
            nc.scalar.dma_start(out=out_ap[img, :, r : r + 2, :], in_=oth[:])


_CACHE = {}


def _get_nc():
    key = "nc"
    if key not in _CACHE:
        nc = bacc.Bacc("TRN2", target_bir_lowering=False, debug=False)
        xp = nc.dram_tensor(
            "xp", [B_LOC, CIN, HP, WP], mybir.dt.float16, kind="ExternalInput"
        ).ap()
        sp = nc.dram_tensor(
            "sp", [B_LOC, CIN, HP, WP], mybir.dt.float16, kind="ExternalInput"
        ).ap()
        w = nc.dram_tensor(
            "w", [P, KH * KW, COUT], mybir.dt.float16, kind="ExternalInput"
        ).ap()
        out = nc.dram_tensor(
            "out", [B_LOC, COUT, H, W], mybir.dt.float16, kind="ExternalOutput"
        ).ap()
        sink = nc.dram_tensor("warm_sink", [1, 4], mybir.dt.float32).ap()
        with tile.TileContext(nc) as tc:
            build_conv(tc, out, xp, sp, w, sink)
        nc.finalize()
        _CACHE[key] = nc
    return _CACHE[key]


def _prep_inputs(x, spline_weights, basis_weights):
    w_spline = spline_weights.sum(axis=-1)  # (COUT, CIN, 3, 3)
    w_cat = np.concatenate([w_spline, basis_weights], axis=1)  # (COUT, 128, 3, 3)
    # -> (cin_cat, tap, cout)
    w_ktm = np.ascontiguousarray(
        w_cat.transpose(1, 2, 3, 0).reshape(P, KH * KW, COUT).astype(np.float16)
    )
    x_pad = np.pad(x, ((0, 0), (0, 0), (1, 1), (1, 1))).astype(np.float16)
    s = (x * (1.0 / (1.0 + np.exp(-x)))).astype(np.float32)  # silu in fp32
    s_pad = np.pad(s, ((0, 0), (0, 0), (1, 1), (1, 1))).astype(np.float16)
    return x_pad, s_pad, w_ktm


def kernel(x, spline_weights, basis_weights, _trace=False, _tmpdir=None):
    x = np.asarray(x, dtype=np.float32)
    spline_weights = np.asarray(spline_weights, dtype=np.float32)
    basis_weights = np.asarray(basis_weights, dtype=np.float32)
    x_pad, s_pad, w_ktm = _prep_inputs(x, spline_weights, basis_weights)
    nc = _get_nc()
    in_maps = [
        {
            "xp": x_pad[B_LOC * c : B_LOC * (c + 1)],
            "sp": s_pad[B_LOC * c : B_LOC * (c + 1)],
            "w": w_ktm,
        }
        for c in range(N_CORES)
    ]
    res = run_bass_kernel_spmd(
        nc, in_maps, list(range(N_CORES)), trace=_trace, tmpdir=_tmpdir
    )
    out = np.concatenate(
        [res.results[c]["out"] for c in range(N_CORES)], axis=0
    ).astype(np.float32)
    if _trace:
        kernel.last_results = res
    return out
